# revision 11
# baseline (speedup 1.0000x reference)
"""AttnBlock (GroupNorm -> single-head attention over 64x64 tokens -> proj -> residual)
for Trainium2, SPMD over 8 NeuronCores.

Sharding: core = batch(4) x query-half(2).  Each core receives x[b] with its
query half rotated to the front (token order along j is permutation-invariant
for softmax-attention and for GroupNorm stats), computes GroupNorm over all
4096 tokens, then a FOLDED attention pipeline for its 2048 query tokens.

Weight folds (exact algebra, done host-side in f64):
  S[j,i] = sum_o q[o,i] k[o,j]; softmax over j is invariant to per-i shifts,
  so S can be computed as h^T . q' with q' = (Wk^T Wq) h + Wk^T bq  -- the
  k conv disappears (h8 is reused as the S lhsT directly).
  out = Wp(V.A) + bp = ((Wp Wv) h).A + (bp + Wp bv)  since sum_j A[j,i] = 1
  -- the vp conv (weights Wp@Wv) directly produces the projected attention
  output; the separate output-projection conv and the fp8 re-quantization of
  the attention output both disappear.

All matmuls run in fp8(e4m3) with DoubleRow perf mode (0.5 cycles/row).
Scales are powers of two and fold away:
  wqk8 = 16*(Wk^T Wq)^T fp8, vp8 = 16*(Wp Wv)^T fp8, h8 = h (scale 1)
  S_psum = 16*(h.q');  et = exp(S/sqrt(C) - ln16) <= ~92 (fp8e4 max 240)
  l_psum = sum_j 16*et = sum_j e^S' via a DoubleRow ones(=16)-matmul
  lrb = 1/l_psum broadcast to 128 partitions by a tiny bf16 ones-matmul
  O_psum = (16vp).(e^S'/16) = sum_j vp.e^S';  y = O_psum*lrb + (x + bp')
Residual uses the bf16 x already in SBUF; y is stored/DMA'd as bf16 and
converted to f32 host-side (adds ~0.2% on top of the 0.45% pipeline error).

Hardware-legality notes (walrus BIR verifier):
  - GPSIMD (Pool) cannot touch PSUM: all psum drains sit on DVE or ACT.
  - Tensor ops may read at most ONE psum operand (lrb goes through SBUF).
  - DoubleRow ldweights requires pair-dim stride % 16 == 0 (all pair strides
    here are 4096/2048/512 bytes).

Schedule: one continuous S->exp stream across all four i-blocks; AV/l trail
by a deep lag on block 0 (8 slots, decaying to 4) so the vp-conv drains
(DVE-bound) spread over ~22 slots; per-block epilogues (recip, lrb bcast,
yt = O*lrb, residual+store on Pool) overlap the next block's exp window.
GroupNorm stats are split DVE (chunks 0,1 + half of 3) / ACT (chunk 2 +
half of 3, Copy/Square accumulate -- same act table as Exp); h8 conversion
is split DVE (2x SBUF mode) / Pool.

Cost-model timing (CoreSim, per core): see test.py; fp8 end-to-end rel err
vs the fp32 reference: ~4.5e-3 (gate 2e-2).
"""

import math
import numpy as np
import ml_dtypes

import concourse.bass as bass
import concourse.mybir as mybir
import concourse.tile as tile

P = 128
C = 512
NCC = C // P          # 4 channel chunks
NP2 = NCC // 2        # 2 channel-chunk pairs (DoubleRow)
HW = 4096             # tokens per batch image
IHALF = 2048          # query tokens per core
NBLK = IHALF // 512   # 4 i-blocks of 512
NJC = HW // P         # 32 j chunks of 128
NJP = NJC // 2        # 16 j-chunk pairs
GS = 16               # channels per group
EPS = 1e-6
WS = 16.0             # host-side weight scale (power of two)
SCALE_S = 1.0 / (WS * math.sqrt(C))
EXP_BIAS = -math.log(16.0)
ONES_VAL = 16.0       # l_psum = sum(e^S') -> lrb = 1/l exactly

F32 = mybir.dt.float32
BF16 = mybir.dt.bfloat16
FP8 = mybir.dt.float8e4
BF = ml_dtypes.bfloat16
E4 = ml_dtypes.float8_e4m3
DR = mybir.MatmulPerfMode.DoubleRow
ALU = mybir.AluOpType
ACTF = mybir.ActivationFunctionType


def _split_excess_waits(nc):
    """walrus in this container accepts only ONE sync-wait per instruction;
    move extra waits onto same-engine NOPs placed immediately before."""
    for fn in nc.m.functions:
        for bb in fn.blocks:
            insts = list(bb.instructions)
            out = []
            changed = False
            for inst in insts:
                si = inst.sync_info
                if si is not None and len(si.on_wait) > 1:
                    waits = list(si.on_wait)
                    for k, w in enumerate(waits[:-1]):
                        nop = mybir.InstNoOp(
                            name=f"{inst.name}-ws{k}",
                            sync_info=mybir.SyncInfo(on_wait=[w], on_update=[]),
                            bass_nofuse=True,
                            engine=inst.engine,
                        )
                        out.append(nop)
                    inst.sync_info = mybir.SyncInfo(
                        on_wait=[waits[-1]], on_update=list(si.on_update)
                    )
                    changed = True
                out.append(inst)
            if changed:
                bb.instructions = out


def build_nc(split_waits=True):
    nc = bass.Bass()

    x_d = nc.declare_dram_parameter("x_bf", [C, HW], BF16, isOutput=False)
    wqk_d = nc.declare_dram_parameter("wqk8", [C, C], FP8, isOutput=False)
    wvp_d = nc.declare_dram_parameter("wvp8", [C, C], FP8, isOutput=False)
    # packed per-channel constants: bqk16, bp', gamma, beta (NCC each), then
    # ind16 (P//GS cols)
    consts_d = nc.declare_dram_parameter("consts", [P, 4 * NCC + P // GS], F32,
                                         isOutput=False)
    bcast16_d = nc.declare_dram_parameter("bcast16", [P // GS, P], F32,
                                          isOutput=False)
    ones8_d = nc.declare_dram_parameter("ones8", [P, 2, 16], FP8, isOutput=False)
    y_d = nc.declare_dram_parameter("yout", [C, IHALF], BF16, isOutput=True)

    with tile.TileContext(nc) as tc:
        with (
            tc.tile_pool(name="w", bufs=1) as wpool,
            tc.tile_pool(name="const", bufs=1) as cpool,
            tc.tile_pool(name="xb", bufs=1) as xpool,
            tc.tile_pool(name="h8p", bufs=1) as hpool,
            tc.tile_pool(name="q8p", bufs=1) as qpool,
            tc.tile_pool(name="v8p", bufs=1) as vpool,
        ):
            wqk8 = wpool.tile([P, NCC, C], FP8, tag="wqk8")
            wvp8 = wpool.tile([P, NCC, C], FP8, tag="wvp8")

            consts = cpool.tile([P, 4 * NCC + P // GS], F32, tag="consts")
            bqk16 = consts[:, 0 * NCC:1 * NCC]
            bppc = consts[:, 1 * NCC:2 * NCC]
            gamma = consts[:, 2 * NCC:3 * NCC]
            beta = consts[:, 3 * NCC:4 * NCC]
            ind16 = consts[:, 4 * NCC:]
            bcast16 = cpool.tile([P // GS, P], F32, tag="bcast16")
            ones8 = cpool.tile([P, 2, 16], FP8, tag="ones8")
            ones_bf = cpool.tile([1, P], BF16, tag="onesbf")
            eps_sb = cpool.tile([P // GS, 1], F32, tag="eps")
            ebias = cpool.tile([P, 1], F32, tag="ebias")

            x_sb = xpool.tile([P, NCC, HW], BF16, tag="x")
            h8 = hpool.tile([P, NCC, HW], FP8, tag="h8")
            q8 = qpool.tile([P, NCC, IHALF], FP8, tag="q8")
            vt8 = vpool.tile([P, NJC, C], FP8, tag="vt8")

            # ---- DMAs.  sync: chunk 0 in eighths (DVE bn_stats chases the
            # arrivals) then chunk 3; scalar: chunk 2 first (its stats run on
            # ACT and gate the critical path); gpsimd: consts, chunk 1, weights.
            egt = HW // 8
            qtr = HW // 4
            for qq in range(4):
                nc.scalar.dma_start(out=x_sb[:, 2, qq * qtr:(qq + 1) * qtr],
                                    in_=x_d[2 * P:3 * P, qq * qtr:(qq + 1) * qtr])
            for qq in range(8):
                nc.sync.dma_start(out=x_sb[:, 0, qq * egt:(qq + 1) * egt],
                                  in_=x_d[0:P, qq * egt:(qq + 1) * egt])
            nc.gpsimd.dma_start(out=consts[:], in_=consts_d[:])
            nc.gpsimd.dma_start(out=bcast16[:], in_=bcast16_d[:])
            nc.gpsimd.dma_start(out=ones8[:], in_=ones8_d[:])
            for qq in range(4):
                nc.gpsimd.dma_start(out=x_sb[:, 1, qq * qtr:(qq + 1) * qtr],
                                    in_=x_d[1 * P:2 * P, qq * qtr:(qq + 1) * qtr])
            for qq in range(4):
                nc.sync.dma_start(out=x_sb[:, 3, qq * qtr:(qq + 1) * qtr],
                                  in_=x_d[3 * P:4 * P, qq * qtr:(qq + 1) * qtr])
            for t, d in ((wqk8, wqk_d), (wvp8, wvp_d)):
                nc.gpsimd.dma_start(out=t[:], in_=d[:].rearrange("(cc p) o -> p cc o", p=P))
            nc.vector.memset(ones_bf[:], 1.0)
            nc.vector.memset(eps_sb[:], EPS)
            nc.vector.memset(ebias[:], EXP_BIAS)
            # load the Sqrt act table right after the dma issues on ACT
            sqwarm = cpool.tile([P // GS, 1], F32, tag="sqwarm")
            nc.scalar.activation(
                out=sqwarm[:], in_=eps_sb[:], func=ACTF.Sqrt, scale=1.0,
            )

            # ====== GroupNorm ======
            # stats: DVE bn_stats chunks 0,1 + first half of 3; ACT Copy/
            # Square accumulate chunk 2 + second half of 3 (quarter-size
            # passes so the per-chunk Sqrt can interleave).
            # h8 = x*sc+sh -> fp8: chunks 0,1 on Pool, chunks 2,3 on DVE
            # (2x SBUF mode).
            with (
                tc.tile_pool(name="gn", bufs=2) as gpool,
                tc.tile_pool(name="gnp", bufs=2, space="PSUM") as gpsum_pool,
            ):
                gpsum = gpsum_pool.tile([P // GS, 2 * NCC], F32, tag="gstat")
                sc_all = gpool.tile([P, NCC], F32, tag="scall")
                sh_all = gpool.tile([P, NCC], F32, tag="shall")

                def finish_chunk(ci, t2):
                    nc.tensor.matmul(
                        gpsum[:, ci * 2:(ci + 1) * 2], lhsT=ind16, rhs=t2[:],
                        start=True, stop=True,
                    )
                    gmr = gpool.tile([P // GS, 2], F32, tag="gmr", name=f"gmr{ci}")
                    nc.vector.tensor_copy(out=gmr[:], in_=gpsum[:, ci * 2:(ci + 1) * 2])
                    mu = gmr[:, 0:1]
                    var = gmr[:, 1:2]
                    tmpv = gpool.tile([P // GS, 1], F32, tag="tmpv")
                    nc.vector.tensor_tensor(tmpv[:], mu, mu, ALU.mult)
                    nc.vector.tensor_tensor(var, var, tmpv[:], ALU.subtract)
                    nc.scalar.activation(
                        out=var, in_=var, func=ACTF.Sqrt, bias=eps_sb[:], scale=1.0,
                    )
                    nc.vector.reciprocal(out=var, in_=var)
                    bpsum = gpsum_pool.tile([P, 2], F32, tag="bc")
                    nc.tensor.matmul(
                        bpsum[:], lhsT=bcast16[:], rhs=gmr[:], start=True, stop=True,
                    )
                    sc = sc_all[:, ci:ci + 1]
                    sh = sh_all[:, ci:ci + 1]
                    nc.vector.tensor_tensor(sc, bpsum[:, 1:2], gamma[:, ci:ci + 1], ALU.mult)
                    nc.vector.tensor_tensor(sh, bpsum[:, 0:1], sc, ALU.mult)
                    nc.vector.tensor_tensor(sh, beta[:, ci:ci + 1], sh, ALU.subtract)
                    # h8 conversion: chunks 0,1 on Pool; 2,3 on DVE (2x mode)
                    if ci in (0, 1):
                        nc.gpsimd.tensor_scalar(
                            out=h8[:, ci, :], in0=x_sb[:, ci, :],
                            scalar1=sc, scalar2=sh, op0=ALU.mult, op1=ALU.add,
                        )
                    else:
                        nc.vector.tensor_scalar(
                            out=h8[:, ci, :], in0=x_sb[:, ci, :],
                            scalar1=sc, scalar2=sh, op0=ALU.mult, op1=ALU.add,
                        )

                def dve_chunk_t2(ci, sls):
                    """bn_stats on DVE over the given 512-col subranges."""
                    stats = gpool.tile([P, len(sls), 6], F32, tag=f"stats{ci}",
                                       name=f"stats{ci}")
                    for k, sg in enumerate(sls):
                        nc.vector.bn_stats(
                            out=stats[:, k, :],
                            in_=x_sb[:, ci, sg * 512:(sg + 1) * 512],
                        )
                    mv = gpool.tile([P, 2], F32, tag="mv", name=f"mv{ci}")
                    nc.vector.bn_aggr(out=mv[:], in_=stats[:])
                    return mv

                def mv_to_t2(mv, t2):
                    # t2 = [mean, mean^2 + var] = [E[x], E[x^2]]
                    nc.vector.tensor_copy(out=t2[:, 0:1], in_=mv[:, 0:1])
                    nc.vector.tensor_tensor(
                        t2[:, 1:2], mv[:, 0:1], mv[:, 0:1], ALU.mult
                    )
                    nc.vector.tensor_add(t2[:, 1:2], t2[:, 1:2], mv[:, 1:2])

                # ACT chunk-2 stats: Copy/Square with accums in ~1us pieces so
                # the tiny per-chunk Sqrts can slot between them (both funcs
                # live in the Exp act table set -> no reloads).  Scratch
                # output lands in vt8 (overwritten by vp drains later).
                c2acc = gpool.tile([P, 8], F32, tag="c2acc")
                scr = vt8[:].rearrange("p jc c -> p (jc c)")
                for hh in range(2):
                    nc.scalar.activation(
                        out=scr[:, hh * 2048:(hh + 1) * 2048],
                        in_=x_sb[:, 2, hh * 2048:(hh + 1) * 2048],
                        func=ACTF.Copy, accum_out=c2acc[:, hh:hh + 1],
                    )
                for hh in range(2):
                    nc.scalar.activation(
                        out=scr[:, 4096 + hh * 2048:4096 + (hh + 1) * 2048],
                        in_=x_sb[:, 2, hh * 2048:(hh + 1) * 2048],
                        func=ACTF.Square, accum_out=c2acc[:, 4 + hh:5 + hh],
                    )

                # DVE: chunk 0 (chasing eighth arrivals), then chunk 1
                mv0 = dve_chunk_t2(0, range(8))
                t20 = gpool.tile([P, 2], F32, tag="t20")
                mv_to_t2(mv0, t20)
                finish_chunk(0, t20)
                mv1 = dve_chunk_t2(1, range(8))
                t21 = gpool.tile([P, 2], F32, tag="t21")
                mv_to_t2(mv1, t21)
                finish_chunk(1, t21)

                # chunk 2: combine ACT accums on DVE (emitted before the c3
                # ACT passes so sqrt2 isn't queued behind them)
                t22 = gpool.tile([P, 2], F32, tag="t22")
                nc.vector.tensor_add(t22[:, 0:1], c2acc[:, 0:1], c2acc[:, 1:2])
                nc.vector.tensor_add(t22[:, 1:2], c2acc[:, 4:5], c2acc[:, 5:6])
                nc.vector.tensor_scalar_mul(t22[:], t22[:], 1.0 / HW)
                finish_chunk(2, t22)

                # chunk 3: DVE bn_stats first half + ACT accums second half
                c3acc = gpool.tile([P, 4], F32, tag="c3acc")
                for hh in range(2):
                    nc.scalar.activation(
                        out=scr[:, 8192 + hh * 1024:8192 + (hh + 1) * 1024],
                        in_=x_sb[:, 3, 2048 + hh * 1024:2048 + (hh + 1) * 1024],
                        func=ACTF.Copy, accum_out=c3acc[:, hh:hh + 1],
                    )
                for hh in range(2):
                    nc.scalar.activation(
                        out=scr[:, 10240 + hh * 1024:10240 + (hh + 1) * 1024],
                        in_=x_sb[:, 3, 2048 + hh * 1024:2048 + (hh + 1) * 1024],
                        func=ACTF.Square, accum_out=c3acc[:, 2 + hh:3 + hh],
                    )

                mv3 = dve_chunk_t2(3, range(4))
                t23 = gpool.tile([P, 2], F32, tag="t23")
                # E[x] = mv3.mean/2 + (c3acc0+c3acc1)/HW
                # E[x^2] = (mv3.var + mv3.mean^2)/2 + (c3acc2+c3acc3)/HW
                nc.vector.tensor_add(t23[:, 0:1], c3acc[:, 0:1], c3acc[:, 1:2])
                nc.vector.tensor_add(t23[:, 1:2], c3acc[:, 2:3], c3acc[:, 3:4])
                nc.vector.tensor_scalar_mul(t23[:], t23[:], 1.0 / HW)
                tm3 = gpool.tile([P, 2], F32, tag="tm3")
                nc.vector.tensor_tensor(tm3[:, 0:1], mv3[:, 0:1], mv3[:, 0:1], ALU.mult)
                nc.vector.tensor_add(tm3[:, 0:1], tm3[:, 0:1], mv3[:, 1:2])
                nc.vector.scalar_tensor_tensor(
                    out=t23[:, 1:2], in0=tm3[:, 0:1], scalar=0.5, in1=t23[:, 1:2],
                    op0=ALU.mult, op1=ALU.add,
                )
                nc.vector.scalar_tensor_tensor(
                    out=t23[:, 0:1], in0=mv3[:, 0:1], scalar=0.5, in1=t23[:, 0:1],
                    op0=ALU.mult, op1=ALU.add,
                )
                finish_chunk(3, t23)

                # preload the Exp activation table after the last Sqrt (input
                # dep on sc_all pins it there despite list scheduling)
                expwarm = gpool.tile([P, 1], F32, tag="expwarm")
                nc.scalar.activation(
                    out=expwarm[:], in_=sc_all[:, 3:4], func=ACTF.Exp, scale=1.0,
                )

            # ====== convs + attention (fused pipeline, all fp8 DoubleRow) =====
            with (
                tc.tile_pool(name="et", bufs=12) as etpool,
                tc.tile_pool(name="lb", bufs=2) as lbpool,
                tc.tile_pool(name="yt", bufs=6) as ytpool,
                tc.tile_pool(name="stp", bufs=3, space="PSUM") as stpool,
                tc.tile_pool(name="oap", bufs=1, space="PSUM") as oapool,
                tc.tile_pool(name="lp", bufs=1, space="PSUM") as lpool,
            ):
                def emit_q(ib, drain_eng):
                    isl = slice(ib * 512, (ib + 1) * 512)
                    for oc in range(NCC):
                        ps = stpool.tile([P, 512], F32, tag="st", name=f"q{ib}{oc}")
                        for p2 in range(NP2):
                            nc.tensor.matmul(
                                ps[:],
                                lhsT=wqk8[:, 2 * p2:2 * p2 + 2, oc * P:(oc + 1) * P],
                                rhs=h8[:, 2 * p2:2 * p2 + 2, isl],
                                start=(p2 == 0), stop=(p2 == NP2 - 1),
                                perf_mode=DR,
                            )
                        # split: half the drains on ACT, half on DVE (parallel)
                        if drain_eng == "act" and oc < 2:
                            nc.scalar.activation(
                                out=q8[:, oc, isl], in_=ps[:], func=ACTF.Identity,
                                bias=bqk16[:, oc:oc + 1], scale=1.0,
                            )
                        else:
                            nc.vector.tensor_scalar(
                                out=q8[:, oc, isl], in0=ps[:],
                                scalar1=bqk16[:, oc:oc + 1], scalar2=None, op0=ALU.add,
                            )

                def emit_v(jc, pool=None, tag="st"):
                    pool = pool or stpool
                    ps = pool.tile([P, 512], F32, tag=tag, name=f"v{jc}")
                    for p2 in range(NP2):
                        nc.tensor.matmul(
                            ps[:],
                            lhsT=h8[:, 2 * p2:2 * p2 + 2, jc * P:(jc + 1) * P],
                            rhs=wvp8[:, 2 * p2:2 * p2 + 2, :],
                            start=(p2 == 0), stop=(p2 == NP2 - 1),
                            perf_mode=DR,
                        )
                    nc.vector.tensor_copy(out=vt8[:, jc, :], in_=ps[:])

                # ---- single S/exp stream across all blocks; AV/l trail by a
                # deep lag on block 0 (vp drains spread out), 4 after.
                slots = [(ib, jp) for ib in range(NBLK) for jp in range(NJP)]
                ets = {}
                opsums = {}
                lpsums = {}

                def emit_s(ib, jp):
                    isl = slice(ib * 512, (ib + 1) * 512)
                    etp = etpool.tile([P, 2, 512], FP8, tag="et",
                                      name=f"et{ib}_{jp}")
                    for par in range(2):
                        jc = 2 * jp + par
                        ps = stpool.tile([P, 512], F32, tag="st",
                                         name=f"s{ib}_{jc}")
                        for p2 in range(NP2):
                            nc.tensor.matmul(
                                ps[:],
                                lhsT=h8[:, 2 * p2:2 * p2 + 2, jc * P:(jc + 1) * P],
                                rhs=q8[:, 2 * p2:2 * p2 + 2, isl],
                                start=(p2 == 0), stop=(p2 == NP2 - 1),
                                perf_mode=DR,
                            )
                        nc.scalar.activation(
                            out=etp[:, par, :], in_=ps[:],
                            func=ACTF.Exp, scale=SCALE_S, bias=ebias[:],
                        )
                    ets[(ib, jp)] = etp

                def emit_av(ib, jp):
                    if jp == 0:
                        opsums[ib] = [
                            oapool.tile([P, 512], F32, tag=f"o{cc}",
                                        name=f"ops{ib}{cc}")
                            for cc in range(NCC)
                        ]
                        lpsums[ib] = lpool.tile([P, 512], F32, tag="l",
                                                name=f"l{ib}")
                    etp = ets.pop((ib, jp))
                    for cc in range(NCC):
                        nc.tensor.matmul(
                            opsums[ib][cc][:],
                            lhsT=vt8[:, 2 * jp:2 * jp + 2, cc * P:(cc + 1) * P],
                            rhs=etp[:],
                            start=(jp == 0), stop=(jp == NJP - 1),
                            perf_mode=DR,
                        )
                    nc.tensor.matmul(
                        lpsums[ib][0:16, :], lhsT=ones8[:], rhs=etp[:],
                        start=(jp == 0), stop=(jp == NJP - 1),
                        perf_mode=DR,
                    )
                    if jp == NJP - 1:
                        finish_block(ib)

                def finish_block(ib):
                    # lrb = 1/sum(e^S') broadcast via ones-matmul; the
                    # broadcast psum reuses the l bank (free after the recip)
                    isl = slice(ib * 512, (ib + 1) * 512)
                    l_bf = lbpool.tile([1, 512], BF16, tag="lbf", name=f"lbf{ib}")
                    with nc.allow_low_precision(reason="1/l bf16; 0.4% on a 6.5%-of-norm term"):
                        nc.vector.reciprocal(out=l_bf[:], in_=lpsums[ib][0:1, :])
                    lrbps = lpool.tile([P, 512], F32, tag="l", name=f"lrb{ib}")
                    nc.tensor.matmul(
                        lrbps[:], lhsT=ones_bf[:], rhs=l_bf[:], start=True, stop=True,
                    )
                    lrb = lbpool.tile([P, 512], BF16, tag="lrbsb", name=f"lrbsb{ib}")
                    nc.vector.tensor_copy(out=lrb[:], in_=lrbps[:])
                    with nc.allow_low_precision(reason="y stored bf16; residual dominates"):
                        for cc in range(NCC):
                            ytm = ytpool.tile([P, 512], BF16, tag="ytm",
                                              name=f"ytm{ib}{cc}")
                            nc.vector.tensor_tensor(
                                ytm[:], opsums[ib][cc][:], lrb[:], ALU.mult
                            )
                            yo = ytpool.tile([P, 512], BF16, tag="yo",
                                             name=f"yo{ib}{cc}")
                            nc.gpsimd.scalar_tensor_tensor(
                                out=yo[:], in0=x_sb[:, cc, isl],
                                scalar=bppc[:, cc:cc + 1], in1=ytm[:],
                                op0=ALU.add, op1=ALU.add,
                            )
                            deng = nc.sync if cc % 2 == 0 else nc.gpsimd
                            deng.dma_start(out=y_d[cc * P:(cc + 1) * P, isl], in_=yo[:])

                # vp-conv emission: 2 in the prologue (ring), jc 2-9 on the
                # l bank (free until AV(0,0) at slot 8), the rest on the ring
                # at 1/slot.
                emit_q(0, "act")
                emit_v(0)
                emit_v(1)
                vp_next = 10

                av_done = 0

                def drain_av(upto):
                    nonlocal av_done
                    while av_done < upto:
                        emit_av(*slots[av_done])
                        av_done += 1

                for s, (ib, jp) in enumerate(slots):
                    emit_s(ib, jp)
                    if s < 8:
                        emit_v(s + 2, pool=lpool, tag="l")
                    if s % 2 == 0 and vp_next < NJC:
                        emit_v(vp_next)
                        vp_next += 1
                    if vp_next < NJC:
                        emit_v(vp_next)
                        vp_next += 1
                    if (ib, jp) == (0, 12):
                        emit_q(1, "vec")
                    if (ib, jp) == (1, 8):
                        emit_q(2, "vec")
                    if (ib, jp) == (2, 8):
                        emit_q(3, "vec")
                    # AV lag: 8 on block 0, 4 after; taper at the very end
                    lag = 8 if s < 16 else (6 if s < 20 else 4)
                    remaining = len(slots) - 1 - s
                    lag = min(lag, remaining + 1)
                    drain_av(max(0, s + 1 - lag))
                drain_av(len(slots))

    if split_waits:
        _split_excess_waits(nc)
    return nc


_NC = None


def _get_nc():
    global _NC
    if _NC is None:
        _NC = build_nc()
    return _NC


def _core0_feed(inputs):
    """Input map for core 0 (batch 0, first query half) — used by test harnesses."""
    maps = _build_in_maps(**inputs)
    return maps[0]


def _build_in_maps(x, gamma, beta, Wq, bq, Wk, bk, Wv, bv, Wp, bp):
    x = np.asarray(x, dtype=np.float32)
    B, c, H, W = x.shape
    assert (B, c, H, W) == (4, C, 64, 64)

    def pc(v):  # [C] -> [P, NCC]
        return np.ascontiguousarray(np.asarray(v, np.float32).reshape(NCC, P).T)

    ind16 = np.zeros((P, P // GS), np.float32)
    ind16[np.arange(P), np.arange(P) // GS] = 1.0 / GS
    bcast16 = np.zeros((P // GS, P), np.float32)
    bcast16[np.arange(P) // GS, np.arange(P)] = 1.0

    Wq64, Wk64, Wv64, Wp64 = [np.asarray(w, np.float64) for w in (Wq, Wk, Wv, Wp)]
    bq64, bv64, bp64 = [np.asarray(b, np.float64) for b in (bq, bv, bp)]
    Wqk = Wk64.T @ Wq64
    bqk = Wk64.T @ bq64
    Wpv = Wp64 @ Wv64
    bp_eff = bp64 + Wp64 @ bv64

    consts = np.concatenate(
        [pc(WS * bqk.astype(np.float32)),
         pc(bp_eff.astype(np.float32)),
         pc(gamma), pc(beta), ind16], axis=1,
    ).astype(np.float32)

    def w8(w):
        return np.ascontiguousarray(WS * np.asarray(w, np.float32).T).astype(E4)

    shared = {
        "wqk8": w8(Wqk), "wvp8": w8(Wpv),
        "consts": np.ascontiguousarray(consts),
        "bcast16": bcast16,
        "ones8": np.full((P, 2, 16), ONES_VAL, E4),
    }

    xf = x.reshape(B, C, HW)
    in_maps = []
    for core in range(8):
        b, half = divmod(core, 2)
        xb = xf[b]
        if half == 0:
            x_bc = xb
        else:
            x_bc = np.concatenate([xb[:, IHALF:], xb[:, :IHALF]], axis=1)
        in_maps.append({"x_bf": np.ascontiguousarray(x_bc).astype(BF), **shared})
    return in_maps


def kernel(x, gamma, beta, Wq, bq, Wk, bk, Wv, bv, Wp, bp):
    nc = _get_nc()
    in_maps = _build_in_maps(x, gamma, beta, Wq, bq, Wk, bk, Wv, bv, Wp, bp)

    from concourse.bass_utils import run_bass_kernel_spmd

    res = run_bass_kernel_spmd(nc, in_maps, list(range(8)))

    B = 4
    out = np.empty((B, C, HW), np.float32)
    for core in range(8):
        b, half = divmod(core, 2)
        out[b, :, half * IHALF:(half + 1) * IHALF] = np.asarray(
            res.results[core]["yout"]).astype(np.float32)
    return out.reshape(B, C, 64, 64)
